# revision 1
# baseline (speedup 1.0000x reference)
"""Trainium2 Bass kernel v2.3 for nn_MCLMask (bipartite Katz / MCL mask).

All-bf16 datapath (host casts inputs to bf16 = input quantization):
  sup: per-shot DMA + DVE shot-sum tree; norms; stationary pre-scaled by
       snr[s] = 10/||mean_s|| (so exp needs no scale)
  qry: streamed in 8 pair-aligned chunks; squares -> ns2q (PE ones-matmul)
       -> sqrt -> partition_broadcast -> 1/x -> query pre-scaled by 1/||q||
       -> bf16 matmuls (PE) -> exp straight from PSUM -> e20/ft/cs ->
       per-pair ct matmuls (ones-col tricks give rs + fs free) -> ctf
  Neumann solve in z-space (3 applications) + PE-transpose normalize.
"""
import sys
import os
import numpy as np
import ml_dtypes

for _p in ("/opt/trn_rl_repo",):
    if os.path.isdir(_p) and _p not in sys.path:
        sys.path.insert(0, _p)

import concourse.bass as bass
import concourse.bacc as bacc
import concourse.mybir as mybir
import concourse.tile as tile
from concourse import masks
from concourse.bass_utils import run_bass_kernel_spmd

F32 = mybir.dt.float32
BF16 = mybir.dt.bfloat16
AX = mybir.AxisListType
OP = mybir.AluOpType
ACTF = mybir.ActivationFunctionType

B_, S_, C_, HW = 4, 25, 640, 10 * 10
NW, KS = 5, 5
Q_ = 75
PP = 38
M = 100
MS = 500
CCH = 5
STIL = 4
SP = 125
FW = 101            # F block: 100 + ones col (-> rs)
EW = 104            # E20 block: 100 + ones col (-> fs row) + pad (4B aligned)
NITER = 2
CHUNKS = [(0, 5), (5, 5), (10, 5), (15, 5), (20, 5), (25, 5), (30, 5), (35, 3)]

_CACHED = {}


def build_nc():
    nc = bacc.Bacc("TRN2", target_bir_lowering=False, debug=False)
    d_sup = nc.declare_dram_parameter("sup", [KS * C_, MS], BF16, isOutput=False)
    d_qry = nc.declare_dram_parameter("qry", [C_, PP * M], BF16, isOutput=False)
    d_out = nc.declare_dram_parameter("out", [PP, M], F32, isOutput=True)
    d_fs = nc.dram_tensor("d_fs", [1, PP * FW], BF16)

    with tile.TileContext(nc) as tc:
        from contextlib import ExitStack
        with ExitStack() as ctx:
            ek = ctx.enter_context
            p_const = ek(tc.tile_pool(name="const", bufs=1))
            p_sup = ek(tc.tile_pool(name="sup", bufs=1))
            p_big = ek(tc.tile_pool(name="big", bufs=1))
            p_qf = ek(tc.tile_pool(name="qf", bufs=8))
            p_ring = ek(tc.tile_pool(name="ring", bufs=2))
            p_tr = ek(tc.tile_pool(name="tr", bufs=3))
            p_small = ek(tc.tile_pool(name="small", bufs=1))
            p_ps = ek(tc.tile_pool(name="ps", bufs=8, space="PSUM"))

            ones128b = p_const.tile([128, 1], BF16)
            nc.vector.memset(ones128b[:], 1.0)
            ident = p_const.tile([128, 128], BF16)
            masks.make_identity(nc, ident[:])

            cs = p_small.tile([SP, STIL * PP], F32)
            csr = p_small.tile([SP, STIL * PP], F32)
            ctf = p_big.tile([FW, PP * FW], BF16)   # rows 0..99 Chat^T, 100 fs
            CW_E = 5 * EW
            CW_F = 5 * FW

            # ---- support: per-shot DMAs + DVE shot-sum tree (bf16) ----
            sup_acc = p_sup.tile([128, CCH * MS], BF16)
            for k in range(KS):
                sraw = p_tr.tile([128, CCH * MS], BF16, tag="supraw")
                nc.sync.dma_start(
                    sraw[:],
                    d_sup[:][C_ * k:C_ * (k + 1), :]
                    .rearrange("(kc p) s -> p kc s", p=128))
                if k == 0:
                    nc.vector.tensor_copy(sup_acc[:], sraw[:])
                else:
                    nc.vector.tensor_tensor(sup_acc[:], sup_acc[:], sraw[:],
                                            op=OP.add)
            sup_sq = p_sup.tile([128, CCH * MS], BF16)
            nc.scalar.activation(sup_sq[:], sup_acc[:], ACTF.Square)
            ns2s_ps = p_ps.tile([1, MS], F32, tag="ps", name="ns2s")
            for k in range(CCH):
                nc.tensor.matmul(ns2s_ps[:], ones128b[:],
                                 sup_sq[:][:, MS * k:MS * (k + 1)],
                                 start=(k == 0), stop=(k == CCH - 1))
            # st uses the shot-sum: snr = 10/||sum|| = exp(-0.5*ln(0.01*ns2s))
            snr_ln = p_small.tile([1, MS], F32)
            nc.scalar.activation(snr_ln[:], ns2s_ps[:], ACTF.Ln, scale=1e-2)
            snr_row = p_small.tile([1, MS], F32)
            nc.scalar.activation(snr_row[:], snr_ln[:], ACTF.Exp, scale=-0.5)
            snr_bc = p_small.tile([128, MS], F32)
            nc.gpsimd.partition_broadcast(snr_bc[:], snr_row[:])
            # stationary pre-scaled by snr along s
            sup_sc = p_sup.tile([128, CCH * MS], BF16)
            for k in range(CCH):
                nc.vector.tensor_tensor(sup_sc[:][:, MS * k:MS * (k + 1)],
                                        sup_acc[:][:, MS * k:MS * (k + 1)],
                                        snr_bc[:], op=OP.mult)

            def emit_phase4(ci, e20c, ftc):
                p0, npair = CHUNKS[ci]
                Wf = npair * FW
                ct_ps = p_ps.tile([EW, 5 * FW], F32, tag="ps", name=f"ct{ci}")
                for i in range(npair):
                    for j in range(STIL):
                        nc.tensor.matmul(
                            ct_ps[:][:, FW * i:FW * (i + 1)],
                            e20c[:][:, CW_E * j + EW * i: CW_E * j + EW * (i + 1)],
                            ftc[:][:, CW_F * j + FW * i: CW_F * j + FW * (i + 1)],
                            start=(j == 0), stop=(j == STIL - 1))
                nc.scalar.activation(
                    ctf[:][0:FW, FW * p0: FW * p0 + Wf],
                    ct_ps[:][0:FW, 0:Wf], ACTF.Copy)

            def emit_post(ci, e10c, e20c, ftc):
                p0, npair = CHUNKS[ci]
                for j in range(STIL):
                    src = e10c[:][:, 500 * j: 500 * j + M * npair]
                    srcr = src.rearrange("s (p m) -> s p m", m=M)
                    dst = e20c[:][:, CW_E * j: CW_E * j + EW * npair]
                    dstr = dst.rearrange("s (p e) -> s p e", e=EW)[:, :, 0:100]
                    nc.vector.tensor_tensor(dstr, srcr, srcr, op=OP.mult)
                    nc.vector.tensor_reduce(
                        cs[:][:, PP * j + p0: PP * j + p0 + npair],
                        srcr, axis=AX.X, op=OP.add)
                cs_sl = cs[:].rearrange("s (j p) -> s j p", j=STIL)[:, :, p0:p0 + npair]
                csr_sl = csr[:].rearrange("s (j p) -> s j p", j=STIL)[:, :, p0:p0 + npair]
                nc.vector.reciprocal_approx_fast(csr_sl, cs_sl)
                for j in range(STIL):
                    src = e10c[:][:, 500 * j: 500 * j + M * npair]
                    srcr = src.rearrange("s (p m) -> s p m", m=M)
                    dst = ftc[:][:, CW_F * j: CW_F * j + FW * npair]
                    dstr = dst.rearrange("s (p r) -> s p r", r=FW)[:, :, 0:100]
                    eng = nc.vector if j < 2 else nc.gpsimd
                    eng.tensor_tensor(
                        dstr, srcr,
                        csr[:][:, PP * j + p0: PP * j + p0 + npair]
                        .unsqueeze(2).broadcast_to((SP, npair, M)),
                        op=OP.mult)

            # ---- B1: load all query chunks + batched norms ----
            qfs = []
            nsq_row = p_small.tile([1, 4000], F32)
            a_all = p_sup.tile([128, 4000], BF16)
            for half in range(2):
                for ci in range(4 * half, 4 * half + 4):
                    p0, npair = CHUNKS[ci]
                    W = npair * M
                    qf = p_qf.tile([128, CCH * 500], BF16, tag="qf",
                                   name=f"qf{ci}")
                    nc.sync.dma_start(
                        qf[:][:, 0:CCH * W],
                        d_qry[:][:, M * p0: M * (p0 + npair)]
                        .rearrange("(kc p) w -> p kc w", p=128))
                    qfs.append(qf)
                    qsq = p_tr.tile([128, CCH * 500], BF16, tag="qsq")
                    nc.scalar.activation(qsq[:][:, 0:CCH * W],
                                         qf[:][:, 0:CCH * W], ACTF.Square)
                    q01 = p_tr.tile([128, 500], BF16, tag="q01")
                    q23 = p_tr.tile([128, 500], BF16, tag="q23")
                    nc.vector.tensor_tensor(q01[:][:, 0:W], qsq[:][:, 0:W],
                                            qsq[:][:, W:2 * W], op=OP.add)
                    nc.vector.tensor_tensor(q23[:][:, 0:W], qsq[:][:, 2 * W:3 * W],
                                            qsq[:][:, 3 * W:4 * W], op=OP.add)
                    ns2q_ps = p_ps.tile([1, 500], F32, tag="ps", name=f"nq{ci}")
                    for ai, asrc in enumerate((q01[:][:, 0:W], q23[:][:, 0:W],
                                               qsq[:][:, 4 * W:5 * W])):
                        nc.tensor.matmul(ns2q_ps[:][:, 0:W], ones128b[:], asrc,
                                         start=(ai == 0), stop=(ai == 2))
                    nc.scalar.activation(nsq_row[:][:, 500 * ci: 500 * ci + W],
                                         ns2q_ps[:][:, 0:W], ACTF.Copy)
                # batched 1/||q|| for this half: exp(-0.5*ln(ns2))
                lo, hi = 2000 * half, 2000 * half + 2000
                ln_row = p_tr.tile([1, 2000], F32, tag="lnrow")
                nc.scalar.activation(ln_row[:], nsq_row[:][:, lo:hi], ACTF.Ln)
                a_row = p_tr.tile([1, 2000], BF16, tag="arow")
                nc.scalar.activation(a_row[:], ln_row[:], ACTF.Exp, scale=-0.5)
                nc.gpsimd.partition_broadcast(a_all[:][:, lo:hi], a_row[:])

            # ---- B2: matmul + softmax + phase4 pipeline ----
            prev = None
            for ci, (p0, npair) in enumerate(CHUNKS):
                W = npair * M
                qf = qfs[ci]
                e10c = p_ring.tile([SP, STIL * 500], BF16, tag="e10c")
                e20c = p_ring.tile([SP, STIL * CW_E], BF16, tag="e20c")
                ftc = p_ring.tile([SP, STIL * CW_F], BF16, tag="ftc")
                nc.vector.memset(
                    e20c[:].rearrange("s (b e) -> s b e", e=EW)[:, :, 100:104], 0.0)
                nc.vector.memset(
                    e20c[:].rearrange("s (b e) -> s b e", e=EW)[:, :, 100:101], 1.0)
                nc.vector.memset(
                    ftc[:].rearrange("s (b r) -> s b r", r=FW)[:, :, 100:101], 1.0)

                qsc = p_tr.tile([128, CCH * 500], BF16, tag="qsc")
                for k in range(CCH):
                    nc.vector.tensor_tensor(
                        qsc[:][:, 500 * k: 500 * k + W],
                        qf[:][:, W * k: W * (k + 1)],
                        a_all[:][:, 500 * ci: 500 * ci + W], op=OP.mult)

                for j in range(STIL):
                    st_ps = p_ps.tile([SP, 500], F32, tag="ps", name=f"st{ci}_{j}")
                    for k in range(CCH):
                        nc.tensor.matmul(
                            st_ps[:][:, 0:W],
                            sup_sc[:][:, MS * k + SP * j: MS * k + SP * (j + 1)],
                            qsc[:][:, 500 * k: 500 * k + W],
                            start=(k == 0), stop=(k == CCH - 1))
                    nc.scalar.activation(
                        e10c[:][:, 500 * j: 500 * j + W], st_ps[:][:, 0:W],
                        ACTF.Exp)
                if prev is not None:
                    emit_phase4(ci - 1, prev[0], prev[1])
                emit_post(ci, e10c, e20c, ftc)
                prev = (e20c, ftc)
            emit_phase4(len(CHUNKS) - 1, prev[0], prev[1])

            # ---- fs roundtrip, rs, solve ----
            nc.sync.dma_start(d_fs[:], ctf[:][M:M + 1, :])
            fs_col = p_small.tile([M, PP], BF16)
            nc.sync.dma_start(
                fs_col[:], d_fs[:].rearrange("o (p r) -> r (o p)", r=FW)[0:M, :])

            rs_f = p_small.tile([M, PP], F32)
            nc.vector.tensor_copy(
                rs_f[:], ctf[:].rearrange("a (p r) -> a p r", r=FW)[0:M, :, 100:101]
                .rearrange("a p o -> a (p o)"))
            rsr = p_small.tile([M, PP], F32)
            nc.vector.reciprocal_approx_fast(rsr[:], rs_f[:])
            qrsr = p_small.tile([M, PP], F32)
            nc.vector.tensor_scalar(qrsr[:], rsr[:], 0.25, None, op0=OP.mult)
            rhsv = p_small.tile([M, PP], F32)
            nc.vector.tensor_scalar(rhsv[:], fs_col[:], 0.5, 1.0,
                                    op0=OP.mult, op1=OP.add)
            qdr = p_small.tile([M, PP], F32)
            nc.vector.tensor_tensor(qdr[:], qrsr[:], rhsv[:], op=OP.mult)
            halffs = p_small.tile([M, PP], F32)
            nc.vector.tensor_scalar(halffs[:], fs_col[:], 0.5, None, op0=OP.mult)
            zt = p_small.tile([M, PP], BF16)
            nc.vector.tensor_copy(zt[:], qdr[:])
            kq_col = p_small.tile([M, PP], F32)
            wtmp = p_small.tile([M, PP], F32)
            HALF = PP // 2
            groups = [(0, HALF), (HALF, PP)]
            for it in range(NITER):
                last = (it == NITER - 1)
                for gi, (lo, hi) in enumerate(groups):
                    w_ps = p_ps.tile([M, HALF], F32, tag="ps", name=f"w{it}_{gi}")
                    for p in range(lo, hi):
                        nc.tensor.matmul(w_ps[:][:, p - lo:p - lo + 1],
                                         ctf[:][0:M, FW * p: FW * p + M],
                                         zt[:][:, p:p + 1], start=True, stop=True)
                    if not last:
                        nc.vector.tensor_tensor(wtmp[:][:, lo:hi], w_ps[:],
                                                qrsr[:][:, lo:hi], op=OP.mult)
                        nc.vector.tensor_tensor(zt[:][:, lo:hi], wtmp[:][:, lo:hi],
                                                qdr[:][:, lo:hi], op=OP.add)
                    else:
                        nc.vector.tensor_tensor(kq_col[:][:, lo:hi], w_ps[:],
                                                halffs[:][:, lo:hi], op=OP.add)

            kq_b = p_small.tile([M, PP], BF16)
            nc.vector.tensor_copy(kq_b[:], kq_col[:])
            kqt_ps = p_ps.tile([PP, M], BF16, tag="ps", name="kqt")
            nc.tensor.transpose(kqt_ps[:], kq_b[:], ident[:][0:M, 0:M])
            ssum = p_small.tile([PP, 1], F32)
            nc.vector.tensor_reduce(ssum[:], kqt_ps[:], axis=AX.X, op=OP.add)
            sinv = p_small.tile([PP, 1], F32)
            nc.vector.reciprocal_approx_fast(sinv[:], ssum[:])
            out_t = p_small.tile([PP, M], F32)
            nc.vector.tensor_scalar(out_t[:], kqt_ps[:], sinv[:], None, op0=OP.mult)
            nc.sync.dma_start(d_out[:], out_t[:])

    nc.compile()
    return nc


def shard_inputs(support_xf, query_xf):
    support_xf = np.asarray(support_xf, dtype=np.float32)
    query_xf = np.asarray(query_xf, dtype=np.float32)
    in_maps = []
    for core in range(8):
        b = core // 2
        half = core % 2
        qs = np.clip(np.arange(half * PP, half * PP + PP), 0, Q_ - 1)
        sup = np.ascontiguousarray(
            support_xf[b].reshape(NW, KS, C_, HW)
            .transpose(1, 2, 0, 3).reshape(KS * C_, NW * HW)
        ).astype(ml_dtypes.bfloat16)
        qry = np.ascontiguousarray(
            query_xf[b, qs].reshape(PP, C_, HW).transpose(1, 0, 2).reshape(C_, PP * M)
        ).astype(ml_dtypes.bfloat16)
        in_maps.append({"sup": sup, "qry": qry})
    return in_maps


def run_sharded(support_xf, query_xf, trace=False, **kw):
    if "nc" not in _CACHED:
        _CACHED["nc"] = build_nc()
    nc = _CACHED["nc"]
    in_maps = shard_inputs(support_xf, query_xf)
    res = run_bass_kernel_spmd(nc, in_maps, core_ids=list(range(8)), trace=trace, **kw)
    b, q = np.asarray(support_xf).shape[0], np.asarray(query_xf).shape[1]
    out = np.zeros((b, q, 1, 10, 10), np.float32)
    for core in range(8):
        bi = core // 2
        half = core % 2
        real = min(PP, q - half * PP)
        o = res.results[core]["out"][:real]
        out[bi, half * PP: half * PP + real] = o.reshape(real, 1, 10, 10)
    return out, res


def kernel(support_xf, query_xf, n_way=5, k_shot=5):
    out, _ = run_sharded(support_xf, query_xf, trace=False)
    return out

